# revision 33
# baseline (speedup 1.0000x reference)
"""Trainium2 Bass kernel v2 for relative-position attention (nn_Attention).

Reference (B=16, C=128, H=W=32, HEADS=4, d=32, N=1024):
    qkv = W_qkv @ x;  S = scale*(q^T k + q^T r), r = rw+rh;  P = softmax(S);
    out = P @ v^T.

v2 design (vs the v1 per-head serial kernel):
  - Data-parallel over batch: 2 batches/core, 8 cores, no collectives.
  - All matmul operands bf16; all folds done on HOST:
      wT[:, q] *= scale * A,  wT[:, k] *= A,  r *= A   with A = 128/ln2,
    so the score psum holds S' = A*S_true = log2(E)*128 — i.e. the bf16
    EXPONENT-FIELD integer of E. This makes exp a single linear op.
  - exp is split across BOTH elementwise engines per stage:
      ACT: native exp with scale=1/A (exact)          -> E bf16  (heads 0,1)
      DVE: Schraudolph: int16(S' + Bc), bitcast bf16  -> E bf16  (heads 2,3)
    Each runs ~1.1-1.3us/stage on its own [128,1024] psum pair tile.
  - S-pass: 4-way ROW-TILED matmuls (K=32/head at array strips 0/32/64/96,
    tile_position=(32h,0)) -> ~4x PE throughput on the d=32 contraction.
  - O-pass: 4-way COL-TILED matmuls (M=32/head at col strips 0/32/64/96)
    accumulate a dense O [128,512]; a second 4-way pass with a [128,32]
    all-ones stationary accumulates the softmax denominator Z (replicated
    over each 32-row band). O is deferred TWO stages behind S so the PE
    never blocks on exp and stays HAM-warm.
  - Normalization (out = O/Z) happens on HOST: device DMAs raw O rows and
    the Z row; numpy divides. (DMA cannot read PSUM, so O is evacuated
    psum->SBUF by ACT/DVE, then DMA'd.)

PSUM budget (8 banks): score pairs [128,1024]x3 rotating (6 banks; the 3rd
buf gives one-stage-ahead pipelining) + O accum [128,512]x2 (2 banks).
The qkv phase borrows the 3 score bufs for q / k / vT psum.
"""

import numpy as np
import ml_dtypes

B, C, H, W = 16, 128, 32, 32
HEADS = 4
D = C // HEADS            # 32
N = H * W                 # 1024
SCALE = float(D) ** -0.5
NCORES = 8
BPC = B // NCORES         # batches per core

A16 = 128.0 / float(np.log(2.0))    # log2(e) * 128: bf16 exponent-field scale
# Schraudolph bias: 127*128 centers the bf16 exponent; C16 centers the
# (1+f)/2^f sawtooth (minimax ~5.51); +0.5 compensates truncating f32->int16.
import os as _os
C16 = float(_os.environ.get("K_C16", "5.51"))
B16 = 127.0 * 128.0 - C16 + float(_os.environ.get("K_RND", "0.5"))
# Number of leading columns of the DVE score pair that ACT exps instead
# (load-balance knob; 0 = even split of heads between engines).
# ACT measured ~1.088ns/col vs DVE ~1.194ns/col: balance wants ~46.
XSPLIT = int(_os.environ.get("K_XSPLIT", "0"))
# Warmup matmul count (HAM LOW->MID ramp inside the input-DMA window).
NWARM = int(_os.environ.get("K_NWARM", "4"))
# Alternate ACT/DVE exp roles per stage: measured WORSE (70.9us vs 66.4);
# keep off.
SWAP = int(_os.environ.get("K_SWAP", "0"))
# HAM keep-warm fillers: matmuls in the PE idle slots between qkv and
# stage 0 (PRE) and in the empty O-slots of stages 0-1 (STAGE). The HAM
# clock ramps slowly (early MMs 600ns vs 380ns mid-kernel); idle cools it.
WF_PRE = int(_os.environ.get("K_WF_PRE", "0"))
WF_STAGE = int(_os.environ.get("K_WF_STAGE", "0"))
# O-pass defer depth in stages (2 = original; 3 gives DVE more slack)
ODEFER = int(_os.environ.get("K_ODEFER", "3"))


def _build_kernel(nc, tc, tile, mybir, x_ap, w_ap, r_ap, out_ap):
    f32 = mybir.dt.float32
    bf16 = mybir.dt.bfloat16
    i16 = mybir.dt.int16
    Exp = mybir.ActivationFunctionType.Exp
    mult = mybir.AluOpType.mult
    add = mybir.AluOpType.add

    const = tc.alloc_tile_pool(name="const", bufs=1)
    sb = tc.alloc_tile_pool(name="sb", bufs=2)
    ep = tc.alloc_tile_pool(name="ep", bufs=5)
    sp = tc.alloc_tile_pool(name="sp", bufs=3, space="PSUM")
    po = tc.alloc_tile_pool(name="po", bufs=1, space="PSUM")

    # --- replicated inputs ---
    # memsets on gpsimd: it is program-loaded earliest and otherwise idle,
    # so the PE warmup isn't gated on the (late-initializing) DVE.
    ones32 = const.tile([128, D], bf16)
    nc.gpsimd.memset(ones32[:], 1.0)
    w_s = const.tile([128, 3 * C], bf16)
    r_s = const.tile([128, N], f32)
    x_first = sb.tile([128, N], bf16, tag="x", name="x0")
    # DMA order: w first (small, unblocks the PE warmup), then x halves
    # (each qkv unit consumes one half), then r (needed only at kp-add).
    nc.sync.dma_start(out=w_s[:], in_=w_ap[:])
    for nf in range(2):
        sl = slice(nf * 512, (nf + 1) * 512)
        nc.sync.dma_start(out=x_first[:, sl], in_=x_ap[0, :, sl])
    for nf in range(2):
        sl = slice(nf * 512, (nf + 1) * 512)
        nc.sync.dma_start(out=r_s[:, sl], in_=r_ap[:, sl])

    def evac(oz_ps, b, ih, isl, unit, split=False):
        # DMA cannot read PSUM: evacuate the merged O|Z tile [128,1024]
        # (O cols 0:512, Z bands cols 512:1024), then ship; host divides.
        # Z ships as ONE partition-strided DMA (rows h*32) instead of 4.
        oz_s = sb.tile([128, 1024], f32, tag="oas", name=f"oz{b}_{ih}")
        z_src = oz_s[:, 512:1024].rearrange(
            "(h d) n -> h d n", h=HEADS)[:, 0, :]
        if split:
            # tail: both engines in parallel; o-DMA fires while z evacs
            nc.scalar.copy(out=oz_s[:, 0:512], in_=oz_ps[:, 0:512])
            nc.sync.dma_start(out=out_ap[b, 0:C, isl], in_=oz_s[:, 0:512])
            nc.vector.tensor_copy(out=oz_s[:, 512:1024],
                                  in_=oz_ps[:, 512:1024])
            nc.sync.dma_start(out=out_ap[b, C:C + HEADS, isl], in_=z_src)
            return
        if unit % 2 == 0:
            nc.scalar.copy(out=oz_s[:], in_=oz_ps[:])
        else:
            nc.vector.tensor_copy(out=oz_s[:], in_=oz_ps[:])
        nc.sync.dma_start(out=out_ap[b, 0:C, isl], in_=oz_s[:, 0:512])
        nc.sync.dma_start(out=out_ap[b, C:C + HEADS, isl], in_=z_src)

    def make_qkv_units(b, x_s, halves=False):
        # Returns (closures, tiles): run the closures (in order) to emit the
        # qkv matmuls + staging; tiles = (q_s, kp_s, vt1) filled by them.
        q_s = sb.tile([128, N], bf16, tag="q", name=f"q{b}")
        kp_s = sb.tile([128, N], bf16, tag="kp", name=f"kp{b}")
        vt1 = sb.tile([128, 8, HEADS, D], bf16, tag="vt1", name=f"vt1_{b}")

        def u_q():
            ps_q = sp.tile([128, N], f32, tag="s", name=f"ps_q{b}")
            for nf in range(2):
                sl = slice(nf * 512, (nf + 1) * 512)
                nc.tensor.matmul(ps_q[:, sl], lhsT=w_s[:, 0:128],
                                 rhs=x_s[:, sl], start=True, stop=True)
            if halves:
                for nf in range(2):
                    hs = slice(nf * 512, (nf + 1) * 512)
                    nc.scalar.copy(out=q_s[:, hs], in_=ps_q[:, hs])
            else:
                nc.scalar.copy(out=q_s[:], in_=ps_q[:])

        def u_k():
            ps_k = sp.tile([128, N], f32, tag="s", name=f"ps_k{b}")
            for nf in range(2):
                sl = slice(nf * 512, (nf + 1) * 512)
                nc.tensor.matmul(ps_k[:, sl], lhsT=w_s[:, 128:256],
                                 rhs=x_s[:, sl], start=True, stop=True)
            if halves:
                for nf in range(2):
                    hs = slice(nf * 512, (nf + 1) * 512)
                    nc.vector.tensor_add(out=kp_s[:, hs], in0=ps_k[:, hs],
                                         in1=r_s[:, hs])
            else:
                nc.vector.tensor_add(out=kp_s[:], in0=ps_k[:], in1=r_s[:])

        def u_v():
            # vT: per j-chunk, out[j, (h,d)] = x_chunk^T @ W_v^T
            ps_v = sp.tile([128, N], f32, tag="s", name=f"ps_v{b}")
            for jc in range(8):
                nc.tensor.matmul(ps_v[:, jc * 128:(jc + 1) * 128],
                                 lhsT=x_s[:, jc * 128:(jc + 1) * 128],
                                 rhs=w_s[:, 256:384], start=True, stop=True)
            nc.scalar.copy(out=vt1[:], in_=ps_v[:].rearrange(
                "p (jc h d) -> p jc h d", jc=8, h=HEADS))

        return [u_q, u_k, u_v], (q_s, kp_s, vt1)

    # batch 1's x is prefetched immediately; its qkv units are injected into
    # batch 0's last stages (one per stage) to avoid the boundary dip.
    x_tiles = [x_first]
    for b in range(1, BPC):
        x_s = sb.tile([128, N], bf16, tag="x", name=f"x{b}")
        for nf in range(2):
            sl = slice(nf * 512, (nf + 1) * 512)
            nc.sync.dma_start(out=x_s[:, sl], in_=x_ap[b, :, sl])
        x_tiles.append(x_s)

    qkv_inject = []
    qkv_tiles = {}

    # HAM warmup: scratch matmuls during the input-DMA window (PE is
    # otherwise idle there) so the clock is at MID by the qkv phase.
    # Uses w_s (first DMA to land) so nothing else gates the PE.
    wp = sp.tile([128, N], f32, tag="s", name="warm_ps")
    for i in range(NWARM):
        nc.tensor.matmul(wp[:, 0:384], lhsT=w_s[:, 0:128],
                         rhs=w_s[:, 0:384], start=True, stop=True)

    pending_o = []
    pending_evac = []
    warm2 = None
    for b in range(BPC):
        if b == 0:
            units, tiles = make_qkv_units(0, x_tiles[0], halves=True)
            for u in units:
                u()
            qkv_tiles[0] = tiles
            # keep-warm fillers in the qkv->stage0 PE idle gap (the PE
            # otherwise waits ~1.3us for q-copy/kp-add staging)
            warm2 = po.tile([128, N], f32, tag="oa", name="warm2")
            for i in range(WF_PRE):
                nc.tensor.matmul(warm2[:, 0:384], lhsT=w_s[:, 0:128],
                                 rhs=w_s[:, 0:384], start=True, stop=True)
        q_s, kp_s, vt1 = qkv_tiles[b]
        if b + 1 < BPC:
            units, tiles = make_qkv_units(b + 1, x_tiles[b + 1])
            qkv_inject.extend(units)
            qkv_tiles[b + 1] = tiles

        # --- attention stages (O-pass deferred one stage for pipelining:
        # the PE issues S(t+1) while ACT/DVE drain exp(t); O(t) then runs
        # without blocking the PE queue on the exp) ---
        for ih in range(2):
            isl = slice(ih * 512, (ih + 1) * 512)
            unit = b * 2 + ih
            oz_ps = po.tile([128, 1024], f32, tag="oa", name=f"oz{b}_{ih}")
            for jc in range(8):
                jsl = slice(jc * 128, (jc + 1) * 128)
                sa = sp.tile([128, N], f32, tag="s", name=f"sa{b}_{ih}_{jc}")
                sd = sp.tile([128, N], f32, tag="s", name=f"sd{b}_{ih}_{jc}")
                for h in (0, 1):
                    nc.tensor.matmul(
                        sa[:, h * 512:(h + 1) * 512],
                        lhsT=kp_s[h * D:(h + 1) * D, jsl],
                        rhs=q_s[h * D:(h + 1) * D, isl],
                        start=True, stop=True, tile_position=(h * D, 0),
                    )
                for h in (2, 3):
                    nc.tensor.matmul(
                        sd[:, (h - 2) * 512:(h - 1) * 512],
                        lhsT=kp_s[h * D:(h + 1) * D, jsl],
                        rhs=q_s[h * D:(h + 1) * D, isl],
                        start=True, stop=True, tile_position=(h * D, 0),
                    )
                if ih == 1 and jc >= 5 and qkv_inject:
                    qkv_inject.pop(0)()
                # last unit: drain O-passes eagerly (defer 1, not 2) so the
                # tail after the final exp is as short as possible
                # O deferred THREE stages behind S: DVE (the slower exp
                # engine) oscillates up to ~1.5 stages behind ACT, so a
                # 2-stage defer makes the PE's O-group wait on ed(t-2).
                # ep bufs=5 covers the extra live E tile.
                last_unit = (b == BPC - 1 and ih == 1)
                thresh = 1 if (last_unit and jc >= 5) else ODEFER
                if len(pending_o) >= thresh:
                    pending_o.pop(0)()
                elif warm2 is not None:
                    # empty O-slot (stages 0-1 of unit 0): keep the PE warm
                    for i in range(WF_STAGE):
                        nc.tensor.matmul(warm2[:, 512:896],
                                         lhsT=w_s[:, 0:128],
                                         rhs=w_s[:, 0:384],
                                         start=True, stop=True)
                ea = ep.tile([128, N], bf16, tag="ea", name=f"ea{b}_{ih}_{jc}")
                ed = ep.tile([128, N], bf16, tag="ed", name=f"ed{b}_{ih}_{jc}")
                # Alternate engines per stage: the psum-buf rotation couples
                # this stage's matmuls to the exp of 1-2 stages ago; swapping
                # which engine exps which tile averages the fast(ACT)/slow
                # (DVE) chains instead of always putting one on the critical
                # 1-stage edge.
                stage = (unit * 8 + jc) if SWAP else 0
                t_act, t_dve = (sa, sd) if stage % 2 == 0 else (sd, sa)
                e_act = ea if t_act is sa else ed
                e_dve = ed if t_act is sa else ea
                nc.scalar.activation(out=e_act[:], in_=t_act[:], func=Exp,
                                     scale=float(1.0 / A16))
                nc.vector.tensor_scalar(
                    out=e_dve[:].bitcast(i16), in0=t_dve[:],
                    scalar1=1.0, scalar2=B16, op0=mult, op1=add,
                )
                if pending_evac:
                    pending_evac.pop(0)()

                def o_pass(jc=jc, oz_ps=oz_ps, ea=ea, ed=ed,
                           b=b, ih=ih, isl=isl, vt1=vt1, unit=unit):
                    esl = [ea[:, 0:512], ea[:, 512:1024],
                           ed[:, 0:512], ed[:, 512:1024]]
                    for h in range(HEADS):
                        nc.tensor.matmul(
                            oz_ps[h * D:(h + 1) * D, 0:512],
                            lhsT=vt1[:, jc, h, :], rhs=esl[h],
                            start=(jc == 0), stop=(jc == 7),
                            tile_position=(0, h * D),
                            skip_group_check=True)
                    for h in range(HEADS):
                        nc.tensor.matmul(
                            oz_ps[h * D:(h + 1) * D, 512:1024],
                            lhsT=ones32[:], rhs=esl[h],
                            start=(jc == 0), stop=(jc == 7),
                            tile_position=(0, h * D),
                            skip_group_check=True)
                    if jc == 7:
                        spl = (unit == 2 * BPC - 1)
                        pending_evac.append(
                            lambda: evac(oz_ps, b, ih, isl, unit, split=spl))

                pending_o.append(o_pass)

    while pending_o:
        pending_o.pop(0)()
    while pending_evac:
        pending_evac.pop(0)()

    for p in (po, sp, ep, sb, const):
        p.release()


def build_nc():
    import concourse.bacc as bacc
    import concourse.tile as tile
    from concourse import mybir

    f32 = mybir.dt.float32
    bf16 = mybir.dt.bfloat16
    nc = bacc.Bacc("TRN2", target_bir_lowering=False, debug=False,
                   num_devices=NCORES)
    x_ap = nc.dram_tensor("x", [BPC, C, N], bf16, kind="ExternalInput").ap()
    w_ap = nc.dram_tensor("wT", [C, 3 * C], bf16, kind="ExternalInput").ap()
    r_ap = nc.dram_tensor("r", [C, N], f32, kind="ExternalInput").ap()
    out_ap = nc.dram_tensor("out", [BPC, C + HEADS, N], f32,
                            kind="ExternalOutput").ap()
    with tile.TileContext(nc) as tc:
        _build_kernel(nc, tc, tile, mybir, x_ap, w_ap, r_ap, out_ap)
    nc.compile()
    return nc


def make_in_maps(x, W_qkv, rw, rh):
    x_ = np.asarray(x, np.float32).reshape(B, C, N)
    x_bf = np.ascontiguousarray(x_).astype(ml_dtypes.bfloat16)
    wT = np.ascontiguousarray(np.asarray(W_qkv, np.float32).T)
    wT[:, 0:C] *= SCALE                # q rows: attention score scale
    wT[:, C:2 * C] *= A16              # k rows: exponent scale
    wT_bf = wT.astype(ml_dtypes.bfloat16)
    rw_ = np.asarray(rw, np.float32).reshape(HEADS, D, 1, W)
    rh_ = np.asarray(rh, np.float32).reshape(HEADS, D, H, 1)
    r = ((rw_ + rh_) * A16).reshape(C, N).astype(np.float32)
    r = np.ascontiguousarray(r)
    return [
        {"x": x_bf[i * BPC:(i + 1) * BPC], "wT": wT_bf, "r": r}
        for i in range(NCORES)
    ]


def _finalize(raw):
    """raw: [B, C+HEADS, N] f32 -> [B, C, H, W]: divide by Z, reshape."""
    o = raw[:, 0:C, :].reshape(-1, HEADS, D, N)
    z = raw[:, C:C + HEADS, :].reshape(-1, HEADS, 1, N)
    return (o / z).reshape(-1, C, H, W).astype(np.float32)


def kernel(x, W_qkv, rw, rh):
    from concourse.bass_utils import run_bass_kernel_spmd

    nc = build_nc()
    in_maps = make_in_maps(x, W_qkv, rw, rh)
    res = None
    for attempt in range(3):
        try:
            res = run_bass_kernel_spmd(nc, in_maps, list(range(NCORES)))
            break
        except Exception:
            if attempt == 2:
                raise
    raw = np.concatenate([np.asarray(r["out"], np.float32)
                          for r in res.results], axis=0)
    return _finalize(raw)



# revision 34
# speedup vs baseline: 1.0400x; 1.0400x over previous
"""Trainium2 Bass kernel v2 for relative-position attention (nn_Attention).

Reference (B=16, C=128, H=W=32, HEADS=4, d=32, N=1024):
    qkv = W_qkv @ x;  S = scale*(q^T k + q^T r), r = rw+rh;  P = softmax(S);
    out = P @ v^T.

v2 design (vs the v1 per-head serial kernel):
  - Data-parallel over batch: 2 batches/core, 8 cores, no collectives.
  - All matmul operands bf16; all folds done on HOST:
      wT[:, q] *= scale * A,  wT[:, k] *= A,  r *= A   with A = 128/ln2,
    so the score psum holds S' = A*S_true = log2(E)*128 — i.e. the bf16
    EXPONENT-FIELD integer of E. This makes exp a single linear op.
  - exp is split across BOTH elementwise engines per stage:
      ACT: native exp with scale=1/A (exact)          -> E bf16  (heads 0,1)
      DVE: Schraudolph: int16(S' + Bc), bitcast bf16  -> E bf16  (heads 2,3)
    Each runs ~1.1-1.3us/stage on its own [128,1024] psum pair tile.
  - S-pass: 4-way ROW-TILED matmuls (K=32/head at array strips 0/32/64/96,
    tile_position=(32h,0)) -> ~4x PE throughput on the d=32 contraction.
  - O-pass: 4-way COL-TILED matmuls (M=32/head at col strips 0/32/64/96)
    accumulate a dense O [128,512]; a second 4-way pass with a [128,32]
    all-ones stationary accumulates the softmax denominator Z (replicated
    over each 32-row band). O is deferred TWO stages behind S so the PE
    never blocks on exp and stays HAM-warm.
  - Normalization (out = O/Z) happens on HOST: device DMAs raw O rows and
    the Z row; numpy divides. (DMA cannot read PSUM, so O is evacuated
    psum->SBUF by ACT/DVE, then DMA'd.)

PSUM budget (8 banks): score pairs [128,1024]x3 rotating (6 banks; the 3rd
buf gives one-stage-ahead pipelining) + O accum [128,512]x2 (2 banks).
The qkv phase borrows the 3 score bufs for q / k / vT psum.
"""

import numpy as np
import ml_dtypes

B, C, H, W = 16, 128, 32, 32
HEADS = 4
D = C // HEADS            # 32
N = H * W                 # 1024
SCALE = float(D) ** -0.5
NCORES = 8
BPC = B // NCORES         # batches per core

A16 = 128.0 / float(np.log(2.0))    # log2(e) * 128: bf16 exponent-field scale
# Schraudolph bias: 127*128 centers the bf16 exponent; C16 centers the
# (1+f)/2^f sawtooth (minimax ~5.51); +0.5 compensates truncating f32->int16.
import os as _os
C16 = float(_os.environ.get("K_C16", "5.51"))
B16 = 127.0 * 128.0 - C16 + float(_os.environ.get("K_RND", "0.5"))
# Number of leading columns of the DVE score pair that ACT exps instead
# (load-balance knob; 0 = even split of heads between engines).
# ACT measured ~1.088ns/col vs DVE ~1.194ns/col: balance wants ~46.
XSPLIT = int(_os.environ.get("K_XSPLIT", "0"))
# Warmup matmul count (HAM LOW->MID ramp inside the input-DMA window).
NWARM = int(_os.environ.get("K_NWARM", "4"))
# Alternate ACT/DVE exp roles per stage: measured WORSE (70.9us vs 66.4);
# keep off.
SWAP = int(_os.environ.get("K_SWAP", "0"))
# HAM keep-warm fillers: matmuls in the PE idle slots between qkv and
# stage 0 (PRE) and in the empty O-slots of stages 0-1 (STAGE). The HAM
# clock ramps slowly (early MMs 600ns vs 380ns mid-kernel); idle cools it.
WF_PRE = int(_os.environ.get("K_WF_PRE", "0"))
WF_STAGE = int(_os.environ.get("K_WF_STAGE", "0"))
# O-pass defer depth in stages (2 = original; 3 gives DVE more slack)
ODEFER = int(_os.environ.get("K_ODEFER", "2"))


def _build_kernel(nc, tc, tile, mybir, x_ap, w_ap, r_ap, out_ap):
    f32 = mybir.dt.float32
    bf16 = mybir.dt.bfloat16
    i16 = mybir.dt.int16
    Exp = mybir.ActivationFunctionType.Exp
    mult = mybir.AluOpType.mult
    add = mybir.AluOpType.add

    const = tc.alloc_tile_pool(name="const", bufs=1)
    sb = tc.alloc_tile_pool(name="sb", bufs=2)
    ep = tc.alloc_tile_pool(name="ep", bufs=5)
    sp = tc.alloc_tile_pool(name="sp", bufs=3, space="PSUM")
    po = tc.alloc_tile_pool(name="po", bufs=1, space="PSUM")

    # --- replicated inputs ---
    # memsets on gpsimd: it is program-loaded earliest and otherwise idle,
    # so the PE warmup isn't gated on the (late-initializing) DVE.
    ones32 = const.tile([128, D], bf16)
    nc.gpsimd.memset(ones32[:], 1.0)
    w_s = const.tile([128, 3 * C], bf16)
    r_s = const.tile([128, N], f32)
    x_first = sb.tile([128, N], bf16, tag="x", name="x0")
    # DMA order: w first (small, unblocks the PE warmup), then x halves
    # (each qkv unit consumes one half), then r (needed only at kp-add).
    nc.sync.dma_start(out=w_s[:], in_=w_ap[:])
    for nf in range(2):
        sl = slice(nf * 512, (nf + 1) * 512)
        nc.sync.dma_start(out=x_first[:, sl], in_=x_ap[0, :, sl])
    for nf in range(2):
        sl = slice(nf * 512, (nf + 1) * 512)
        nc.sync.dma_start(out=r_s[:, sl], in_=r_ap[:, sl])

    def evac(oz_ps, b, ih, isl, unit, split=False):
        # DMA cannot read PSUM: evacuate the merged O|Z tile [128,1024]
        # (O cols 0:512, Z bands cols 512:1024), then ship; host divides.
        # Z ships as ONE partition-strided DMA (rows h*32) instead of 4.
        oz_s = sb.tile([128, 1024], f32, tag="oas", name=f"oz{b}_{ih}")
        z_src = oz_s[:, 512:1024].rearrange(
            "(h d) n -> h d n", h=HEADS)[:, 0, :]
        if split:
            # tail: both engines in parallel; o-DMA fires while z evacs
            nc.scalar.copy(out=oz_s[:, 0:512], in_=oz_ps[:, 0:512])
            nc.sync.dma_start(out=out_ap[b, 0:C, isl], in_=oz_s[:, 0:512])
            nc.vector.tensor_copy(out=oz_s[:, 512:1024],
                                  in_=oz_ps[:, 512:1024])
            nc.sync.dma_start(out=out_ap[b, C:C + HEADS, isl], in_=z_src)
            return
        if unit % 2 == 0:
            nc.scalar.copy(out=oz_s[:], in_=oz_ps[:])
        else:
            nc.vector.tensor_copy(out=oz_s[:], in_=oz_ps[:])
        nc.sync.dma_start(out=out_ap[b, 0:C, isl], in_=oz_s[:, 0:512])
        nc.sync.dma_start(out=out_ap[b, C:C + HEADS, isl], in_=z_src)

    def make_qkv_units(b, x_s, halves=False):
        # Returns (closures, tiles): run the closures (in order) to emit the
        # qkv matmuls + staging; tiles = (q_s, kp_s, vt1) filled by them.
        q_s = sb.tile([128, N], bf16, tag="q", name=f"q{b}")
        kp_s = sb.tile([128, N], bf16, tag="kp", name=f"kp{b}")
        vt1 = sb.tile([128, 8, HEADS, D], bf16, tag="vt1", name=f"vt1_{b}")

        def u_q():
            ps_q = sp.tile([128, N], f32, tag="s", name=f"ps_q{b}")
            for nf in range(2):
                sl = slice(nf * 512, (nf + 1) * 512)
                nc.tensor.matmul(ps_q[:, sl], lhsT=w_s[:, 0:128],
                                 rhs=x_s[:, sl], start=True, stop=True)
            if halves:
                for nf in range(2):
                    hs = slice(nf * 512, (nf + 1) * 512)
                    nc.scalar.copy(out=q_s[:, hs], in_=ps_q[:, hs])
            else:
                nc.scalar.copy(out=q_s[:], in_=ps_q[:])

        def u_k():
            ps_k = sp.tile([128, N], f32, tag="s", name=f"ps_k{b}")
            for nf in range(2):
                sl = slice(nf * 512, (nf + 1) * 512)
                nc.tensor.matmul(ps_k[:, sl], lhsT=w_s[:, 128:256],
                                 rhs=x_s[:, sl], start=True, stop=True)
            if halves:
                for nf in range(2):
                    hs = slice(nf * 512, (nf + 1) * 512)
                    nc.vector.tensor_add(out=kp_s[:, hs], in0=ps_k[:, hs],
                                         in1=r_s[:, hs])
            else:
                nc.vector.tensor_add(out=kp_s[:], in0=ps_k[:], in1=r_s[:])

        def u_v():
            # vT: per j-chunk, out[j, (h,d)] = x_chunk^T @ W_v^T
            ps_v = sp.tile([128, N], f32, tag="s", name=f"ps_v{b}")
            for jc in range(8):
                nc.tensor.matmul(ps_v[:, jc * 128:(jc + 1) * 128],
                                 lhsT=x_s[:, jc * 128:(jc + 1) * 128],
                                 rhs=w_s[:, 256:384], start=True, stop=True)
            nc.scalar.copy(out=vt1[:], in_=ps_v[:].rearrange(
                "p (jc h d) -> p jc h d", jc=8, h=HEADS))

        return [u_q, u_k, u_v], (q_s, kp_s, vt1)

    # batch 1's x is prefetched immediately; its qkv units are injected into
    # batch 0's last stages (one per stage) to avoid the boundary dip.
    x_tiles = [x_first]
    for b in range(1, BPC):
        x_s = sb.tile([128, N], bf16, tag="x", name=f"x{b}")
        for nf in range(2):
            sl = slice(nf * 512, (nf + 1) * 512)
            nc.sync.dma_start(out=x_s[:, sl], in_=x_ap[b, :, sl])
        x_tiles.append(x_s)

    qkv_inject = []
    qkv_tiles = {}

    # HAM warmup: scratch matmuls during the input-DMA window (PE is
    # otherwise idle there) so the clock is at MID by the qkv phase.
    # Uses w_s (first DMA to land) so nothing else gates the PE.
    wp = sp.tile([128, N], f32, tag="s", name="warm_ps")
    for i in range(NWARM):
        nc.tensor.matmul(wp[:, 0:384], lhsT=w_s[:, 0:128],
                         rhs=w_s[:, 0:384], start=True, stop=True)

    pending_o = []
    pending_evac = []
    warm2 = None
    for b in range(BPC):
        if b == 0:
            units, tiles = make_qkv_units(0, x_tiles[0], halves=True)
            for u in units:
                u()
            qkv_tiles[0] = tiles
            # keep-warm fillers in the qkv->stage0 PE idle gap (the PE
            # otherwise waits ~1.3us for q-copy/kp-add staging)
            warm2 = po.tile([128, N], f32, tag="oa", name="warm2")
            for i in range(WF_PRE):
                nc.tensor.matmul(warm2[:, 0:384], lhsT=w_s[:, 0:128],
                                 rhs=w_s[:, 0:384], start=True, stop=True)
        q_s, kp_s, vt1 = qkv_tiles[b]
        if b + 1 < BPC:
            units, tiles = make_qkv_units(b + 1, x_tiles[b + 1])
            qkv_inject.extend(units)
            qkv_tiles[b + 1] = tiles

        # --- attention stages (O-pass deferred one stage for pipelining:
        # the PE issues S(t+1) while ACT/DVE drain exp(t); O(t) then runs
        # without blocking the PE queue on the exp) ---
        for ih in range(2):
            isl = slice(ih * 512, (ih + 1) * 512)
            unit = b * 2 + ih
            oz_ps = po.tile([128, 1024], f32, tag="oa", name=f"oz{b}_{ih}")
            for jc in range(8):
                jsl = slice(jc * 128, (jc + 1) * 128)
                sa = sp.tile([128, N], f32, tag="s", name=f"sa{b}_{ih}_{jc}")
                sd = sp.tile([128, N], f32, tag="s", name=f"sd{b}_{ih}_{jc}")
                for h in (0, 1):
                    nc.tensor.matmul(
                        sa[:, h * 512:(h + 1) * 512],
                        lhsT=kp_s[h * D:(h + 1) * D, jsl],
                        rhs=q_s[h * D:(h + 1) * D, isl],
                        start=True, stop=True, tile_position=(h * D, 0),
                    )
                for h in (2, 3):
                    nc.tensor.matmul(
                        sd[:, (h - 2) * 512:(h - 1) * 512],
                        lhsT=kp_s[h * D:(h + 1) * D, jsl],
                        rhs=q_s[h * D:(h + 1) * D, isl],
                        start=True, stop=True, tile_position=(h * D, 0),
                    )
                if ih == 1 and jc >= 5 and qkv_inject:
                    qkv_inject.pop(0)()
                # last unit: drain O-passes eagerly (defer 1, not 2) so the
                # tail after the final exp is as short as possible
                # O deferred THREE stages behind S: DVE (the slower exp
                # engine) oscillates up to ~1.5 stages behind ACT, so a
                # 2-stage defer makes the PE's O-group wait on ed(t-2).
                # ep bufs=5 covers the extra live E tile.
                last_unit = (b == BPC - 1 and ih == 1)
                thresh = 1 if (last_unit and jc >= 5) else ODEFER
                if len(pending_o) >= thresh:
                    pending_o.pop(0)()
                elif warm2 is not None:
                    # empty O-slot (stages 0-1 of unit 0): keep the PE warm
                    for i in range(WF_STAGE):
                        nc.tensor.matmul(warm2[:, 512:896],
                                         lhsT=w_s[:, 0:128],
                                         rhs=w_s[:, 0:384],
                                         start=True, stop=True)
                ea = ep.tile([128, N], bf16, tag="ea", name=f"ea{b}_{ih}_{jc}")
                ed = ep.tile([128, N], bf16, tag="ed", name=f"ed{b}_{ih}_{jc}")
                # Alternate engines per stage: the psum-buf rotation couples
                # this stage's matmuls to the exp of 1-2 stages ago; swapping
                # which engine exps which tile averages the fast(ACT)/slow
                # (DVE) chains instead of always putting one on the critical
                # 1-stage edge.
                stage = (unit * 8 + jc) if SWAP else 0
                t_act, t_dve = (sa, sd) if stage % 2 == 0 else (sd, sa)
                e_act = ea if t_act is sa else ed
                e_dve = ed if t_act is sa else ea
                nc.scalar.activation(out=e_act[:], in_=t_act[:], func=Exp,
                                     scale=float(1.0 / A16))
                nc.vector.tensor_scalar(
                    out=e_dve[:].bitcast(i16), in0=t_dve[:],
                    scalar1=1.0, scalar2=B16, op0=mult, op1=add,
                )
                if pending_evac:
                    pending_evac.pop(0)()

                def o_pass(jc=jc, oz_ps=oz_ps, ea=ea, ed=ed,
                           b=b, ih=ih, isl=isl, vt1=vt1, unit=unit):
                    esl = [ea[:, 0:512], ea[:, 512:1024],
                           ed[:, 0:512], ed[:, 512:1024]]
                    for h in range(HEADS):
                        nc.tensor.matmul(
                            oz_ps[h * D:(h + 1) * D, 0:512],
                            lhsT=vt1[:, jc, h, :], rhs=esl[h],
                            start=(jc == 0), stop=(jc == 7),
                            tile_position=(0, h * D),
                            skip_group_check=True)
                    for h in range(HEADS):
                        nc.tensor.matmul(
                            oz_ps[h * D:(h + 1) * D, 512:1024],
                            lhsT=ones32[:], rhs=esl[h],
                            start=(jc == 0), stop=(jc == 7),
                            tile_position=(0, h * D),
                            skip_group_check=True)
                    if jc == 7:
                        spl = (unit == 2 * BPC - 1)
                        pending_evac.append(
                            lambda: evac(oz_ps, b, ih, isl, unit, split=spl))

                pending_o.append(o_pass)

    while pending_o:
        pending_o.pop(0)()
    while pending_evac:
        pending_evac.pop(0)()

    for p in (po, sp, ep, sb, const):
        p.release()


def build_nc():
    import concourse.bacc as bacc
    import concourse.tile as tile
    from concourse import mybir

    f32 = mybir.dt.float32
    bf16 = mybir.dt.bfloat16
    nc = bacc.Bacc("TRN2", target_bir_lowering=False, debug=False,
                   num_devices=NCORES)
    x_ap = nc.dram_tensor("x", [BPC, C, N], bf16, kind="ExternalInput").ap()
    w_ap = nc.dram_tensor("wT", [C, 3 * C], bf16, kind="ExternalInput").ap()
    r_ap = nc.dram_tensor("r", [C, N], f32, kind="ExternalInput").ap()
    out_ap = nc.dram_tensor("out", [BPC, C + HEADS, N], f32,
                            kind="ExternalOutput").ap()
    with tile.TileContext(nc) as tc:
        _build_kernel(nc, tc, tile, mybir, x_ap, w_ap, r_ap, out_ap)
    nc.compile()
    return nc


def make_in_maps(x, W_qkv, rw, rh):
    x_ = np.asarray(x, np.float32).reshape(B, C, N)
    x_bf = np.ascontiguousarray(x_).astype(ml_dtypes.bfloat16)
    wT = np.ascontiguousarray(np.asarray(W_qkv, np.float32).T)
    wT[:, 0:C] *= SCALE                # q rows: attention score scale
    wT[:, C:2 * C] *= A16              # k rows: exponent scale
    wT_bf = wT.astype(ml_dtypes.bfloat16)
    rw_ = np.asarray(rw, np.float32).reshape(HEADS, D, 1, W)
    rh_ = np.asarray(rh, np.float32).reshape(HEADS, D, H, 1)
    r = ((rw_ + rh_) * A16).reshape(C, N).astype(np.float32)
    r = np.ascontiguousarray(r)
    return [
        {"x": x_bf[i * BPC:(i + 1) * BPC], "wT": wT_bf, "r": r}
        for i in range(NCORES)
    ]


def _finalize(raw):
    """raw: [B, C+HEADS, N] f32 -> [B, C, H, W]: divide by Z, reshape."""
    o = raw[:, 0:C, :].reshape(-1, HEADS, D, N)
    z = raw[:, C:C + HEADS, :].reshape(-1, HEADS, 1, N)
    return (o / z).reshape(-1, C, H, W).astype(np.float32)


def kernel(x, W_qkv, rw, rh):
    from concourse.bass_utils import run_bass_kernel_spmd

    nc = build_nc()
    in_maps = make_in_maps(x, W_qkv, rw, rh)
    res = None
    for attempt in range(3):
        try:
            res = run_bass_kernel_spmd(nc, in_maps, list(range(NCORES)))
            break
        except Exception:
            if attempt == 2:
                raise
    raw = np.concatenate([np.asarray(r["out"], np.float32)
                          for r in res.results], axis=0)
    return _finalize(raw)

